# revision 1
# baseline (speedup 1.0000x reference)
"""Trainium2 Bass kernel for nn_DetectionHead (NMS detection head).

Computes, for x[8, 2048, 2048] f32:
    xp  = relu(x - eps)
    xm  = 3x3 hole-excluded neighborhood max of xp (zero padding)
    out = xp * (x > xm)

Sharding: batch (8 images) across the 8 NeuronCores, data parallel.
The host pads each image with a 1-pixel zero border ([2050, 2050]) so the
device kernel needs no boundary special-casing (pad 0 <= eps behaves exactly
like the reference's zero-padded relu pool).

Per-core layout: full-width row bands. Band t covers image rows
[512t, 512t+512); partition p holds padded rows 512t+4p .. 512t+4p+5 (4 data
rows + 2 halo rows) x the full padded width, so the whole 3x3 stencil is
free-dim-local (no cross-partition access — compute-engine APs must start at
partition 0/32/64/96 on TRN2, so partition-shifted operands are not an
option), and every DMA chunk is a full ~8.2KB row (small-chunk stores were
measured 2x slower).  The pipeline per band is six 2-input vector ops:

    v  = max(row-above, row-below)            TT max  (vertical hole pair)
    c  = max(v, center)                       TT max  (3-tall column max)
    m1 = max(c@col-1, c@col+1)                TT max  (in-place onto c)
    q  = max(m1, v, 2*eps)                    STT (max,max), in-place
    g  = (q - eps) < x                        STT (sub,is_lt), in-place
    out = (x - eps) * g                       STT (sub,mult)

All six layers run on the DVE: it is the only 2-input elementwise engine this
container's walrus accepts (GPSIMD tensor_tensor/scalar_tensor_tensor and
custom-DVE ops are rejected by its ISA checks — the GP_COLS_* = 0 switches
keep the column-split machinery available for a matched compiler).  ~55% of
the ~230us/core is the DVE chain; loads/stores overlap underneath it.

Exactness (bit-exact vs the reference):
 - relu is monotone, so max_i relu(x_i - eps) == relu(max_i x_i - eps); the
   pool runs on raw x.
 - x > relu(m - eps) == (x > m - eps) & (x > 0); and with the out-factor
   relu(x - eps) the (x > 0) term can be strengthened to (x > eps).
 - max(m - eps, eps) == max(m, 2*eps) - eps holds exactly in fp32 (2*eps is
   exact, and both sides reduce to the same rounded value by monotonicity of
   rounding), so g == (x > m - eps) & (x > eps) exactly.
 - when g = 1, x > eps so (x - eps) == relu(x - eps); when g = 0 the product
   is exactly 0.
"""

import numpy as np

import concourse.bacc as bacc
import concourse.mybir as mybir
import concourse.tile as tile
from concourse import bass_utils
from concourse.ap import AP

EPS = 0.01
EPS2 = float(np.float32(0.01) * 2)  # exact 2*fl(eps)
B, H, W = 8, 2048, 2048
HP2, WP2 = H + 2, W + 2   # host-padded image
P = 128                   # SBUF partitions
R = H // P                # data rows per partition (16)
S = R + 2                 # row slots incl halo
RB = 4                    # rows per partition per band
BAND_H = RB * P           # 512 image rows per band
NBAND = H // BAND_H       # 4 bands
SB = RB + 2               # row slots incl halo
XT_BUFS = 2
F32 = mybir.dt.float32
MX = mybir.AluOpType.max

# Columns of each layer run on GPSIMD (Pool engine) instead of DVE.
GP_COLS_VC = 0
GP_COLS_M = 0
GP_COLS_G = 0


def _split_tt(nc, out3, in0_3, in1_3, gp_cols, op=MX):
    """tensor_tensor, with the rightmost gp_cols columns on GPSIMD."""
    wtot = out3.shape[2]
    cut = wtot - gp_cols
    if gp_cols <= 0:
        nc.vector.tensor_tensor(out=out3, in0=in0_3, in1=in1_3, op=op)
        return
    nc.vector.tensor_tensor(
        out=out3[:, :, 0:cut], in0=in0_3[:, :, 0:cut], in1=in1_3[:, :, 0:cut], op=op
    )
    nc.gpsimd.tensor_tensor(
        out=out3[:, :, cut:wtot],
        in0=in0_3[:, :, cut:wtot],
        in1=in1_3[:, :, cut:wtot],
        op=op,
    )


def _split_stt(nc, out3, in0_3, scalar, in1_3, op0, op1, gp_cols):
    """scalar_tensor_tensor, with the rightmost gp_cols columns on GPSIMD."""
    wtot = out3.shape[2]
    cut = wtot - gp_cols
    if gp_cols <= 0:
        nc.vector.scalar_tensor_tensor(
            out=out3, in0=in0_3, scalar=scalar, in1=in1_3, op0=op0, op1=op1
        )
        return
    nc.vector.scalar_tensor_tensor(
        out=out3[:, :, 0:cut],
        in0=in0_3[:, :, 0:cut],
        scalar=scalar,
        in1=in1_3[:, :, 0:cut],
        op0=op0,
        op1=op1,
    )
    nc.gpsimd.scalar_tensor_tensor(
        out=out3[:, :, cut:wtot],
        in0=in0_3[:, :, cut:wtot],
        scalar=scalar,
        in1=in1_3[:, :, cut:wtot],
        op0=op0,
        op1=op1,
    )


def _emit_pipeline(nc, tc, x_d, o_d, out_row_stride, out_offset0, mode="full"):
    """Full-width row-band pipeline: band t covers image rows
    [512t, 512t+512); partition p holds rows 512t+4p-1 .. 512t+4p+4 (4 data
    rows + 2 halo rows) x the full padded width.  Every DMA chunk is a full
    (padded) row, ~8.2KB contiguous."""
    do_load = mode in ("full", "dmaonly", "loadonly")
    do_store = mode in ("full", "dmaonly", "storeonly")
    do_compute = mode in ("full", "nodma")
    with (
        tc.tile_pool(name="iox", bufs=XT_BUFS) as iox,
        tc.tile_pool(name="work", bufs=1) as wp,
        tc.tile_pool(name="ioo", bufs=1) as ioo,
    ):
        HALF = W // 2  # 1024
        for t in range(NBAND):
            first, last = (t == 0), (t == NBAND - 1)
            v = wp.tile([P, RB, WP2], F32, tag="v")
            c = wp.tile([P, RB, WP2], F32, tag="c")
            cw = c[:, :, 0:W]  # in-place m1/q/g all land here
            o = ioo.tile([P, RB, W], F32, tag="o")

            if first and do_compute:
                # Band 0 is column-split into two half-width chains fed by
                # two half loads (separate tiles in the same tag's slots),
                # so the DVE starts after ~half the first load instead of
                # the whole of it.
                xtL = iox.tile([P, SB, HALF + 2], F32, tag="xt")
                xtR = iox.tile([P, SB, HALF + 2], F32, tag="xt")
                for xth, cb in ((xtL, 0), (xtR, HALF)):
                    if do_load:
                        nc.sync.dma_start(
                            out=xth[:],
                            in_=AP(
                                x_d.tensor,
                                t * BAND_H * WP2 + cb,
                                [[RB * WP2, P], [WP2, SB], [1, HALF + 2]],
                            ),
                        )
                    else:
                        nc.vector.memset(xth[:], 0.25)
                for xth, cb in ((xtL, 0), (xtR, HALF)):
                    vh = v[:, :, cb : cb + HALF + 2]
                    ch = c[:, :, cb : cb + HALF + 2]
                    cwh = cw[:, :, cb : cb + HALF]
                    xch = xth[:, 1 : RB + 1, 1 : HALF + 1]
                    nc.vector.tensor_tensor(
                        out=vh, in0=xth[:, 0:RB, :], in1=xth[:, 2:SB, :], op=MX
                    )
                    nc.vector.tensor_tensor(
                        out=ch, in0=vh, in1=xth[:, 1 : RB + 1, :], op=MX
                    )
                    nc.vector.tensor_tensor(
                        out=cwh,
                        in0=c[:, :, cb : cb + HALF],
                        in1=c[:, :, cb + 2 : cb + HALF + 2],
                        op=MX,
                    )
                    nc.vector.scalar_tensor_tensor(
                        out=cwh, in0=cwh, scalar=EPS2,
                        in1=v[:, :, cb + 1 : cb + HALF + 1], op0=MX, op1=MX,
                    )
                    nc.vector.scalar_tensor_tensor(
                        out=cwh, in0=cwh, scalar=EPS, in1=xch,
                        op0=mybir.AluOpType.subtract, op1=mybir.AluOpType.is_lt,
                    )
                    nc.vector.scalar_tensor_tensor(
                        out=o[:, :, cb : cb + HALF], in0=xch, scalar=EPS, in1=cwh,
                        op0=mybir.AluOpType.subtract, op1=mybir.AluOpType.mult,
                    )
                if do_store:
                    nc.sync.dma_start(
                        out=AP(
                            o_d.tensor,
                            out_offset0 + t * BAND_H * out_row_stride,
                            [[RB * out_row_stride, P], [out_row_stride, RB], [1, W]],
                        ),
                        in_=o[:, :, 0:W],
                    )
                continue

            xt = iox.tile([P, SB, WP2], F32, tag="xt")
            if do_load:
                nc.sync.dma_start(
                    out=xt[:],
                    in_=AP(
                        x_d.tensor,
                        t * BAND_H * WP2,
                        [[RB * WP2, P], [WP2, SB], [1, WP2]],
                    ),
                )
            else:
                nc.vector.memset(xt[:], 0.25)
            xc = xt[:, 1 : RB + 1, 1 : W + 1]  # center view [P, RB, W]

            if do_compute:
                _split_tt(nc, v[:], xt[:, 0:RB, :], xt[:, 2:SB, :], GP_COLS_VC)
                _split_tt(nc, c[:], v[:], xt[:, 1 : RB + 1, :], GP_COLS_VC)
                # m1 = max(c@col-1, c@col+1), onto c (write index trails both
                # read indices in stream order, so the overlap is safe)
                _split_tt(nc, cw, c[:, :, 0:W], c[:, :, 2:WP2], GP_COLS_M)
                # q = max(m1, v, 2*eps): the 2eps clamp makes the g compare
                # below imply x > eps, so no final relu is needed.
                _split_stt(
                    nc, cw, cw, EPS2, v[:, :, 1 : W + 1], MX, MX, GP_COLS_M
                )
                # g = (q - eps) < x  (== (x > m - eps) & (x > eps), exactly)
                _split_stt(
                    nc, cw, cw, EPS, xc,
                    mybir.AluOpType.subtract, mybir.AluOpType.is_lt, GP_COLS_G,
                )
                if not last:
                    _split_stt(
                        nc, o[:, :, 0:W], xc, EPS, cw,
                        mybir.AluOpType.subtract, mybir.AluOpType.mult, GP_COLS_G,
                    )
            else:
                nc.vector.tensor_copy(out=o[:, :, 0:W], in_=xc)

            if do_compute and last:
                # Last band: the final op + store are column-split so the
                # left store overlaps the right half's compute (tail trim).
                for cb in (0, HALF):
                    nc.vector.scalar_tensor_tensor(
                        out=o[:, :, cb : cb + HALF],
                        in0=xt[:, 1 : RB + 1, 1 + cb : 1 + cb + HALF],
                        scalar=EPS,
                        in1=cw[:, :, cb : cb + HALF],
                        op0=mybir.AluOpType.subtract,
                        op1=mybir.AluOpType.mult,
                    )
                    if do_store:
                        nc.sync.dma_start(
                            out=AP(
                                o_d.tensor,
                                out_offset0 + t * BAND_H * out_row_stride + cb,
                                [[RB * out_row_stride, P], [out_row_stride, RB], [1, HALF]],
                            ),
                            in_=o[:, :, cb : cb + HALF],
                        )
            elif do_store:
                nc.sync.dma_start(
                    out=AP(
                        o_d.tensor,
                        out_offset0 + t * BAND_H * out_row_stride,
                        [[RB * out_row_stride, P], [out_row_stride, RB], [1, W]],
                    ),
                    in_=o[:, :, 0:W],
                )


def _build_program():
    nc = bacc.Bacc(
        "TRN2",
        target_bir_lowering=False,
        debug=False,
        enable_asserts=False,
        num_devices=B,
    )
    x_d = nc.dram_tensor("x", [HP2, WP2], F32, kind="ExternalInput").ap()
    o_d = nc.dram_tensor("out", [H, W], F32, kind="ExternalOutput").ap()
    with tile.TileContext(nc) as tc:
        _emit_pipeline(nc, tc, x_d, o_d, W, 0)
    nc.compile()
    return nc


def _build_timing_program(niter=1, mode="full"):
    """Same pipeline repeated `niter` times by a device-side loop, writing
    out as [HP2, WP2].  One execute performs niter full passes, so
    (wall(niter) - wall(1)) / (niter - 1) isolates device time from the
    (identical) transfer cost.  Border cells of out are never written."""
    nc = bacc.Bacc(
        "TRN2",
        target_bir_lowering=False,
        debug=False,
        enable_asserts=False,
        num_devices=B,
    )
    di = nc.dram_tensor("x", [1, 8], F32, kind="ExternalInput").ap()
    do = nc.dram_tensor("out", [1, 8], F32, kind="ExternalOutput").ap()
    # the working image lives in Internal DRAM scratch (contents irrelevant
    # for timing); external I/O is a tiny dummy so transfers are ~free.
    x_d = nc.dram_tensor("xi", [HP2, WP2], F32, kind="Internal").ap()
    o_d = nc.dram_tensor("oi", [HP2, WP2], F32, kind="Internal").ap()
    with tile.TileContext(nc) as tc:
        with tc.tile_pool(name="dummy", bufs=1) as dp:
            dt = dp.tile([1, 8], F32, tag="dummy")
            nc.sync.dma_start(out=dt[:], in_=di[:])
            nc.sync.dma_start(out=do[:], in_=dt[:])
        if niter == 1:
            _emit_pipeline(nc, tc, x_d, o_d, WP2, WP2 + 1, mode)
        else:
            with tc.For_i(0, niter, 1):
                _emit_pipeline(nc, tc, x_d, o_d, WP2, WP2 + 1, mode)
    nc.compile()
    return nc


_NC = None


def _get_program():
    global _NC
    if _NC is None:
        _NC = _build_program()
    return _NC


def kernel(x: np.ndarray) -> np.ndarray:
    x = np.asarray(x, dtype=np.float32)
    assert x.shape == (B, H, W), x.shape
    xpad = np.zeros((B, HP2, WP2), dtype=np.float32)
    xpad[:, 1 : H + 1, 1 : W + 1] = x
    nc = _get_program()
    in_maps = [{"x": xpad[i]} for i in range(B)]
    res = bass_utils.run_bass_kernel_spmd(nc, in_maps, core_ids=list(range(B)))
    return np.stack([r["out"] for r in res.results], axis=0).astype(np.float32)

